# revision 1
# baseline (speedup 1.0000x reference)
"""Trainium2 Bass kernel for the CurvedAssociativeMemory fixed-point iteration.

Computes, for `steps` iterations:
    s <- sign(s @ (J + J^T) + h + kappa * softmax(s, axis=-1))

Strategy: data-parallel over the batch dim across 8 NeuronCores (512 rows
per core), J replicated and streamed from HBM each step.  All matmuls are
native fp32 with K accumulated in ascending 128-row chunks in PSUM, which
bit-matches the XLA lowering of the jax reference on this hardware.  The
softmax is computed in the natural layout with the same op sequence XLA
emits (max-subtract, ACT-table exp, free-dim reduce_sum, DVE reciprocal +
multiply), so the full pipeline tracks the reference to within a few ulps.
"""

import numpy as np

N = 4096          # feature dim
B = 4096          # total batch
N_CORES = 8
B_SH = B // N_CORES   # 512 batch rows per core
P = 128               # partitions
NCHUNK = 256          # matmul moving free-dim per chunk
KO = N // P           # 32 k-tiles
NO = N // NCHUNK      # 8 n-chunks
BT = B_SH // P        # 4 batch tiles per core

# tuning knobs (overridable before _build for experiments)
REPEAT = 1  # timing only: run the whole step body REPEAT times via a HW loop
JPOOL_BUFS = 4
SCRATCH_BUFS = 2
PSUM_BUFS = 8


def _build(steps: int, kappa: float, has_h: bool):
    NCHUNK_ = NCHUNK; NO_ = N // NCHUNK_
    import concourse.bass as bass
    import concourse.tile as tile
    import concourse.mybir as mybir
    from concourse import bacc
    from concourse.masks import make_identity

    F32 = mybir.dt.float32
    AF = mybir.ActivationFunctionType

    nc = bacc.Bacc(None)
    s_in = nc.dram_tensor("s", [B_SH, N], F32, kind="ExternalInput")
    j_in = nc.dram_tensor("J", [N, N], F32, kind="ExternalInput")
    h_in = nc.dram_tensor("h", [N], F32, kind="ExternalInput") if has_h else None
    out = nc.dram_tensor("out", [B_SH, N], F32, kind="ExternalOutput")

    with tile.TileContext(nc) as tc:
        with (
            tc.tile_pool(name="persist", bufs=1) as persist,
            tc.tile_pool(name="jpool", bufs=JPOOL_BUFS) as jpool,
            tc.tile_pool(name="scratch", bufs=SCRATCH_BUFS) as scratch,
            tc.tile_pool(name="stats", bufs=1) as stats,
            tc.tile_pool(name="psum", bufs=PSUM_BUFS, space="PSUM") as psum,
        ):
            ident = persist.tile([P, P], F32, tag="ident", name="ident")
            make_identity(nc, ident)

            # persistent state: c in natural layout, 4 tiles of [128, N]
            c = [persist.tile([P, N], F32, tag=f"c{bt}", name=f"c{bt}") for bt in range(BT)]
            for bt in range(BT):
                nc.sync.dma_start(out=c[bt], in_=s_in.ap()[bt * P:(bt + 1) * P, :])

            # transposed state: cT, 32 tiles of [128, B_SH]
            cT = [persist.tile([P, B_SH], F32, tag=f"t{k}", name=f"t{k}") for k in range(KO)]

            h_bc = None
            if has_h:
                h_bc = persist.tile([P, N], F32, tag="hb", name="hb")
                h_ap = h_in.ap()
                nc.sync.dma_start(
                    out=h_bc,
                    in_=bass.AP(tensor=h_ap.tensor, offset=h_ap.offset,
                                ap=[[0, P], [1, N]]),
                )

            mx = [stats.tile([P, 1], F32, tag=f"mx{bt}", name=f"mx{bt}") for bt in range(BT)]
            rS = [stats.tile([P, 1], F32, tag=f"rS{bt}", name=f"rS{bt}") for bt in range(BT)]

            def emit_steps():
                for _step in range(steps):
                    # ---- phase A: transpose c -> cT, softmax stats per b-tile ----
                    # k-major so cT[k] completes early and the k=0 matmuls can
                    # start while later k-tiles are still transposing.
                    for k in range(KO):
                        for bt in range(BT):
                            ps_t = psum.tile([P, NCHUNK_], F32, tag="pb", name="ps_t")[:, :P]
                            nc.tensor.transpose(ps_t, c[bt][:, k * P:(k + 1) * P], ident)
                            nc.vector.tensor_copy(
                                out=cT[k][:, bt * P:(bt + 1) * P], in_=ps_t)

                    for bt in range(BT):
                        et = scratch.tile([P, N], F32, tag="et", name="et")
                        nc.vector.reduce_max(out=mx[bt], in_=c[bt],
                                             axis=mybir.AxisListType.X)
                        nc.vector.tensor_scalar_sub(out=et, in0=c[bt], scalar1=mx[bt])
                        nc.scalar.activation(out=et, in_=et, func=AF.Exp)
                        ssum = stats.tile([P, 1], F32, tag="ssum", name="ssum")
                        nc.vector.reduce_sum(out=ssum, in_=et,
                                             axis=mybir.AxisListType.X)
                        nc.vector.reciprocal(out=rS[bt], in_=ssum)

                    # ---- phase B: matmul + epilogue per n-chunk ----
                    for n in range(NO_):
                        pm_t = [psum.tile([P, NCHUNK_], F32, tag="pb", name="pm")
                                for _ in range(BT)]
                        for k in range(KO):
                            jt = jpool.tile([P, NCHUNK_], F32, tag="jt", name="jt")
                            nc.sync.dma_start(
                                out=jt,
                                in_=j_in.ap()[k * P:(k + 1) * P,
                                              n * NCHUNK_:(n + 1) * NCHUNK_])
                            for bt in range(BT):
                                nc.tensor.matmul(
                                    pm_t[bt],
                                    cT[k][:, bt * P:(bt + 1) * P],
                                    jt,
                                    start=(k == 0), stop=(k == KO - 1))
                        nsl = slice(n * NCHUNK_, (n + 1) * NCHUNK_)
                        for bt in range(BT):
                            m_sl = pm_t[bt]
                            u = scratch.tile([P, NCHUNK_], F32, tag="u", name="u")
                            if has_h:
                                nc.vector.tensor_add(out=u, in0=m_sl, in1=h_bc[:, nsl])
                            q = scratch.tile([P, NCHUNK_], F32, tag="q", name="q")
                            nc.vector.tensor_scalar_sub(out=q, in0=c[bt][:, nsl],
                                                        scalar1=mx[bt])
                            nc.scalar.activation(out=q, in_=q, func=AF.Exp)
                            nc.vector.tensor_scalar_mul(out=q, in0=q, scalar1=rS[bt])
                            nc.scalar.mul(out=q, in_=q, mul=float(kappa))
                            if has_h:
                                nc.vector.tensor_add(out=u, in0=u, in1=q)
                            else:
                                nc.vector.tensor_add(out=u, in0=m_sl, in1=q)
                            nc.scalar.activation(out=c[bt][:, nsl], in_=u, func=AF.Sign)


            if REPEAT > 1:
                with tc.For_i(0, REPEAT, 1):
                    emit_steps()
            else:
                emit_steps()

            for bt in range(BT):
                nc.sync.dma_start(out=out.ap()[bt * P:(bt + 1) * P, :], in_=c[bt])

    nc.finalize()
    return nc


LAST_RESULTS = None  # BassKernelResults from the most recent kernel() call
LAST_NC = None       # finalized Bass module from the most recent kernel() call


def kernel(s, J, h, kappa, steps):
    import os
    from concourse.bass_utils import run_bass_kernel_spmd

    s = np.ascontiguousarray(np.asarray(s, dtype=np.float32))
    J = np.asarray(J, dtype=np.float32)
    h = np.asarray(h, dtype=np.float32)
    kappa_f = float(np.asarray(kappa))
    steps_i = int(np.asarray(steps))

    Jsym = np.ascontiguousarray(J + J.T)
    has_h = bool(np.any(h))

    nc = _build(steps_i, kappa_f, has_h)
    global LAST_NC
    LAST_NC = nc

    in_maps = []
    for i in range(N_CORES):
        m = {"s": np.ascontiguousarray(s[i * B_SH:(i + 1) * B_SH]), "J": Jsym}
        if has_h:
            m["h"] = h
        in_maps.append(m)

    trace = os.environ.get("CAM_TRACE", "") == "1"
    res = run_bass_kernel_spmd(nc, in_maps, core_ids=list(range(N_CORES)),
                               trace=trace)
    global LAST_RESULTS
    LAST_RESULTS = res
    out = np.concatenate([r["out"] for r in res.results], axis=0)
    return out.astype(np.float32, copy=False)


if __name__ == "__main__":
    rng = np.random.default_rng(0)
    s = rng.standard_normal((B, N)).astype(np.float32)
    J0 = (0.01 * rng.standard_normal((N, N))).astype(np.float32)
    J = ((J0 + J0.T) / 2).astype(np.float32)
    out = kernel(s=s, J=J, h=np.zeros(N, np.float32),
                 kappa=np.float32(0.2), steps=3)
    print(out.shape, np.unique(out, return_counts=True))

